# revision 6
# baseline (speedup 1.0000x reference)
"""Noisy top-k (k=2) router for Trainium2, data-parallel over 8 NeuronCores.

Math: for each row of x = logits + noise with top-2 values v1 >= v2, the
top-2 softmax weights are sigmoid(+-(v1-v2)), and for any element
    out = (x >= v2) * sigmoid(2x - (v1+v2))
equals the scattered result exactly (selection compares run on exact fp32
values, so this matches the reference whenever no row has a v2 == v3 tie).

v2 engine split (vs the v1 baseline, which was DVE+ACT bound by 256 tiny
per-row ACTIVATEs at 333ns and mask/mult passes):
  - sigmoid is batched: u = x - (v1+v2)/2 (one full-width pass) feeds ONE
    big ACT sigmoid per chunk with scale=2.0 (per-row bias folded into u),
    instead of 16 per-row biased ACTIVATEs per subtile.
  - final combine is out = sig * mask (mask in {0,1}; GpSimd lacks min/is_ge).
  - passes are distributed: MAX8 + smalls + mask on DVE, u on GpSimd,
    min alternating DVE/GpSimd, adds via SWDGE accum-DMA (CCE) for most
    chunks with the ramp/tail chunks added on DVE.
  - loads issue on the SP HWDGE ring (nc.sync), stores on the ACT ring
    (nc.scalar) so blocked stores never stall loads.
This walrus codegen allows only ONE sync-wait per instruction; the
_legalize_waits post-pass splits any excess into standalone EventSemaphore
instructions (which hold two).
"""

import time

import numpy as np

import concourse.bass as bass
import concourse.mybir as mybir
from concourse.tile import TileContext
from concourse.bass_utils import run_bass_kernel_spmd

B = 262144
E = 64
N_CORES = 8
B_CORE = B // N_CORES  # 32768 rows per core

P = 128  # SBUF partitions
ROWS_PP = B_CORE // P  # 256 rows per partition

# chunk sizes in rows-per-partition; small chunks at the ends shorten the
# pipeline ramp and the store tail
CHUNK_ROWS = (16, 16, 32, 32, 32, 32, 32, 32, 16, 16)
assert sum(CHUNK_ROWS) == ROWS_PP

# per-chunk add engine: 'dve' TT add (ramp chunk: SWDGE has ~1us latency
# and the DVE is idle early), else SWDGE accum-add on the DMA CCE
ADD_DVE_CHUNKS = (0,)
# per-chunk engine for the final min: True -> DVE, False -> GpSimd
MIN_ON_DVE = (False, True, False, True, False, True, False, True, False, True)

_CACHE = {}

# test.py introspection: BassKernelResults of the most recent run
LAST_RESULT = None


def _legalize_waits(nc: "bass.Bass") -> None:
    """This walrus codegen accepts at most ONE sync-wait per instruction
    (two on EventSemaphore). Tile's wait assigner can emit more; split the
    excess into standalone EventSemaphore waits placed immediately before
    the instruction on the same engine (identical semantics: the engine
    blocks there instead)."""
    n = 0
    for fnb in nc.m.functions[0].blocks:
        out = []
        for inst in fnb.instructions:
            si = inst.sync_info
            cap = 2 if isinstance(inst, mybir.InstEventSemaphore) else 1
            if si is not None and len(si.on_wait) > cap:
                waits = list(si.on_wait)
                extra, keep = waits[:-cap], waits[-cap:]
                for c in range(0, len(extra), 2):
                    n += 1
                    out.append(
                        mybir.InstEventSemaphore(
                            name=f"EVW-{n}",
                            engine=inst.engine,
                            sync_info=mybir.SyncInfo(
                                on_wait=extra[c : c + 2], on_update=[]
                            ),
                        )
                    )
                inst.sync_info = mybir.SyncInfo(
                    on_wait=keep, on_update=list(si.on_update)
                )
            out.append(inst)
        fnb.instructions = out


def _build_nc() -> bass.Bass:
    nc = bass.Bass()
    f32 = mybir.dt.float32

    lg = nc.dram_tensor("logits", [B_CORE, E], f32, kind="ExternalInput")
    nz = nc.dram_tensor("noise", [B_CORE, E], f32, kind="ExternalInput")
    out = nc.dram_tensor("out", [B_CORE, E], f32, kind="ExternalOutput")

    # partition-major: partition p owns ROWS_PP contiguous DRAM rows, so a
    # chunk of rc rows/partition is one rc*256B-contiguous descriptor per
    # partition
    lgv = lg[:].rearrange("(p r) e -> p r e", p=P)
    nzv = nz[:].rearrange("(p r) e -> p r e", p=P)
    outv = out[:].rearrange("(p r) e -> p r e", p=P)

    NCH = len(CHUNK_ROWS)
    offs = [0]
    for rc in CHUNK_ROWS:
        offs.append(offs[-1] + rc)

    with TileContext(nc) as tc:
        with (
            tc.tile_pool(name="big", bufs=3) as big_pool,
            tc.tile_pool(name="sm", bufs=3) as sm_pool,
        ):

            def issue_loads(c):
                rc = CHUNK_ROWS[c]
                sl = slice(offs[c], offs[c] + rc)
                x = big_pool.tile([P, rc, E], f32, tag=f"x{rc}")
                if c in ADD_DVE_CHUNKS:
                    lgt = big_pool.tile([P, rc, E], f32, tag=f"lg{rc}", bufs=1)
                    nzt = big_pool.tile([P, rc, E], f32, tag=f"nz{rc}", bufs=1)
                    nc.sync.dma_start(out=lgt, in_=lgv[:, sl])
                    nc.sync.dma_start(out=nzt, in_=nzv[:, sl])
                    nc.vector.tensor_tensor(
                        out=x, in0=lgt, in1=nzt, op=mybir.AluOpType.add
                    )
                else:
                    nc.sync.dma_start(out=x, in_=lgv[:, sl])
                    nc.gpsimd.dma_start(
                        out=x, in_=nzv[:, sl], accum_op=mybir.AluOpType.add
                    )
                return x

            # stage state: per-chunk tiles flowing through the pipeline
            xs = [None] * NCH
            pend = [None] * NCH  # (sg, m, ot, c) awaiting min+store

            def flush_min(c):
                """emit min+store for chunk c (delayed one iteration so the
                DVE/GpSimd queue head never waits on ACT's sigmoid)"""
                sg, m, ot = pend[c]
                eng = nc.vector if MIN_ON_DVE[c] else nc.gpsimd
                eng.tensor_tensor(
                    out=ot, in0=sg, in1=m, op=mybir.AluOpType.mult
                )
                rc = CHUNK_ROWS[c]
                sl = slice(offs[c], offs[c] + rc)
                nc.scalar.dma_start(out=outv[:, sl], in_=ot)

            xs[0] = issue_loads(0)
            for c in range(NCH):
                rc = CHUNK_ROWS[c]
                if c + 1 < NCH:
                    xs[c + 1] = issue_loads(c + 1)

                x = xs[c]
                v8 = sm_pool.tile([P, rc, 8], f32, tag=f"v8{rc}")
                for r in range(rc):
                    nc.vector.max(out=v8[:, r, :], in_=x[:, r, :])

                # sh = (v1 + v2)/2
                s = sm_pool.tile([P, rc], f32, tag=f"s{rc}")
                sh = sm_pool.tile([P, rc], f32, tag=f"sh{rc}")
                nc.vector.tensor_tensor(
                    out=s,
                    in0=v8[:, :, 0],
                    in1=v8[:, :, 1],
                    op=mybir.AluOpType.add,
                )
                nc.vector.tensor_scalar_mul(out=sh, in0=s, scalar1=0.5)

                # m = (x >= v2b), exact fp32 compare  [DVE]
                m = big_pool.tile([P, rc, E], f32, tag=f"m{rc}")
                nc.vector.tensor_tensor(
                    out=m,
                    in0=x,
                    in1=v8[:, :, 1].to_broadcast([P, rc, E]),
                    op=mybir.AluOpType.is_ge,
                )

                # u = x - shb  [GpSimd]
                u = big_pool.tile([P, rc, E], f32, tag=f"u{rc}")
                nc.gpsimd.tensor_tensor(
                    out=u,
                    in0=x,
                    in1=sh.to_broadcast([P, rc, E]),
                    op=mybir.AluOpType.subtract,
                )

                # sig = sigmoid(2u)  [ACT]
                sg = big_pool.tile([P, rc, E], f32, tag=f"sg{rc}")
                nc.scalar.activation(
                    out=sg,
                    in_=u,
                    func=mybir.ActivationFunctionType.Sigmoid,
                    scale=2.0,
                )
                ot = big_pool.tile([P, rc, E], f32, tag=f"ot{rc}")
                pend[c] = (sg, m, ot)

                if c >= 1:
                    flush_min(c - 1)
            flush_min(NCH - 1)

    _legalize_waits(nc)
    return nc


def _get_nc() -> bass.Bass:
    if "nc" not in _CACHE:
        _CACHE["nc"] = _build_nc()
    return _CACHE["nc"]


def kernel(logits: np.ndarray, noise: np.ndarray) -> np.ndarray:
    global LAST_RESULT
    logits = np.ascontiguousarray(np.asarray(logits), dtype=np.float32)
    noise = np.ascontiguousarray(np.asarray(noise), dtype=np.float32)
    assert logits.shape == (B, E) and noise.shape == (B, E)

    lg_shards = np.split(logits, N_CORES, axis=0)
    nz_shards = np.split(noise, N_CORES, axis=0)
    in_maps = [
        {"logits": lg_shards[i], "noise": nz_shards[i]} for i in range(N_CORES)
    ]

    try:
        res = run_bass_kernel_spmd(
            _get_nc(), in_maps, core_ids=list(range(N_CORES))
        )
    except Exception:
        # transient NRT device errors have been observed right after a
        # crashed run; one retry clears them
        time.sleep(5)
        res = run_bass_kernel_spmd(
            _get_nc(), in_maps, core_ids=list(range(N_CORES))
        )
    LAST_RESULT = res
    return np.concatenate([r["out"] for r in res.results], axis=0)


# revision 8
# speedup vs baseline: 1.0639x; 1.0639x over previous
"""Noisy top-k (k=2) router for Trainium2, data-parallel over 8 NeuronCores.

Math: for each row of x = logits + noise with top-2 values v1 >= v2, the
top-2 softmax weights are sigmoid(+-(v1-v2)), and for any element
    out = (x >= v2) * sigmoid(2x - (v1+v2))
equals the scattered result exactly (selection compares run on exact fp32
values, so this matches the reference whenever no row has a v2 == v3 tie).

Engine split (hybrid, tuned against measured per-engine rates:
DVE ~1.1ns/elem, GpSimd TT ~2.0ns/elem, ACT ~0.85ns/elem + 333ns per
per-row ACTIVATE):
  - MAX8 top-8 per row: DVE only (~41us/core, the fixed floor).
  - mask m = (x >= v2_bcast): DVE (exact fp32 compare).
  - sigmoid, two routes per chunk balancing ACT vs DVE/GpSimd:
      route A: u = x - (v1+v2)/2 on GpSimd, then ONE big ACT sigmoid
               (scale=2.0) per chunk;
      route B: per-row ACTIVATE sigmoid with bias=-(v1+v2) (scale=2.0)
               entirely on ACT - no u pass at all.
  - combine out = sig * m: GpSimd mostly (DVE for a few chunks).
  - adds: SWDGE accum-DMA on the CCE (chunk 0 on DVE for the ramp).
  - smalls (s*0.5 / s*-1): ACT Copy with scale (DVE tensor_scalar was
    measured at ~1us per call here, ACT Copy is ~0.3us).
  - loads on the SP HWDGE ring (nc.sync), stores on the ACT ring
    (nc.scalar) so blocked stores never stall loads.
This walrus codegen allows only ONE sync-wait per instruction; the
_legalize_waits post-pass splits any excess into standalone EventSemaphore
instructions (which hold two).
"""

import time

import numpy as np

import concourse.bass as bass
import concourse.mybir as mybir
from concourse.tile import TileContext
from concourse.bass_utils import run_bass_kernel_spmd

B = 262144
E = 64
N_CORES = 8
B_CORE = B // N_CORES  # 32768 rows per core

P = 128  # SBUF partitions
ROWS_PP = B_CORE // P  # 256 rows per partition

# per-chunk config: (rows_per_partition, route, add_eng, m_eng, comb_eng)
#   route: 'A' big-sigmoid (u on GpSimd + 1 ACT), 'B' per-row ACT sigmoid
#   add_eng: 'dve' TT add | 'swdge' accum-DMA
#   m_eng / comb_eng: 'dve' | 'gp'
CHUNKS = (
    (16, "B", "dve", "dve", "dve"),
    (16, "B", "swdge", "dve", "gp"),
    (32, "B", "swdge", "dve", "gp"),
    (32, "B", "swdge", "dve", "gp"),
    (32, "A", "swdge", "dve", "gp"),
    (32, "A", "swdge", "dve", "gp"),
    (32, "A", "swdge", "dve", "gp"),
    (32, "A", "swdge", "dve", "gp"),
    (16, "B", "swdge", "dve", "gp"),
    (16, "B", "swdge", "dve", "dve"),
)
assert sum(c[0] for c in CHUNKS) == ROWS_PP

_CACHE = {}

# test.py introspection: BassKernelResults of the most recent run
LAST_RESULT = None


def _legalize_waits(nc: "bass.Bass") -> None:
    """This walrus codegen accepts at most ONE sync-wait per instruction
    (two on EventSemaphore). Tile's wait assigner can emit more; split the
    excess into standalone EventSemaphore waits placed immediately before
    the instruction on the same engine (identical semantics: the engine
    blocks there instead)."""
    n = 0
    for fnb in nc.m.functions[0].blocks:
        out = []
        for inst in fnb.instructions:
            si = inst.sync_info
            cap = 2 if isinstance(inst, mybir.InstEventSemaphore) else 1
            if si is not None and len(si.on_wait) > cap:
                waits = list(si.on_wait)
                extra, keep = waits[:-cap], waits[-cap:]
                for c in range(0, len(extra), 2):
                    n += 1
                    out.append(
                        mybir.InstEventSemaphore(
                            name=f"EVW-{n}",
                            engine=inst.engine,
                            sync_info=mybir.SyncInfo(
                                on_wait=extra[c : c + 2], on_update=[]
                            ),
                        )
                    )
                inst.sync_info = mybir.SyncInfo(
                    on_wait=keep, on_update=list(si.on_update)
                )
            out.append(inst)
        fnb.instructions = out


def _build_nc() -> bass.Bass:
    nc = bass.Bass()
    f32 = mybir.dt.float32

    lg = nc.dram_tensor("logits", [B_CORE, E], f32, kind="ExternalInput")
    nz = nc.dram_tensor("noise", [B_CORE, E], f32, kind="ExternalInput")
    out = nc.dram_tensor("out", [B_CORE, E], f32, kind="ExternalOutput")

    # partition-major: partition p owns ROWS_PP contiguous DRAM rows, so a
    # chunk of rc rows/partition is one rc*256B-contiguous descriptor per
    # partition
    lgv = lg[:].rearrange("(p r) e -> p r e", p=P)
    nzv = nz[:].rearrange("(p r) e -> p r e", p=P)
    outv = out[:].rearrange("(p r) e -> p r e", p=P)

    NCH = len(CHUNKS)
    offs = [0]
    for c in CHUNKS:
        offs.append(offs[-1] + c[0])

    TT = mybir.AluOpType
    ENG = {"dve": None, "gp": None}  # filled after nc exists

    with TileContext(nc) as tc:
        ENG = {"dve": nc.vector, "gp": nc.gpsimd}
        with (
            tc.tile_pool(name="big", bufs=3) as big_pool,
            tc.tile_pool(name="sm", bufs=3) as sm_pool,
        ):

            def issue_loads(c):
                rc, route, add_eng, m_eng, comb_eng = CHUNKS[c]
                sl = slice(offs[c], offs[c] + rc)
                x = big_pool.tile([P, rc, E], f32, tag=f"x{rc}")
                if add_eng == "dve":
                    lgt = big_pool.tile([P, rc, E], f32, tag=f"lg{rc}", bufs=1)
                    nzt = big_pool.tile([P, rc, E], f32, tag=f"nz{rc}", bufs=1)
                    nc.sync.dma_start(out=lgt, in_=lgv[:, sl])
                    nc.sync.dma_start(out=nzt, in_=nzv[:, sl])
                    nc.vector.tensor_tensor(
                        out=x, in0=lgt, in1=nzt, op=TT.add
                    )
                else:
                    nc.sync.dma_start(out=x, in_=lgv[:, sl])
                    nc.gpsimd.dma_start(
                        out=x, in_=nzv[:, sl], accum_op=TT.add
                    )
                return x

            xs = [None] * NCH
            pend = [None] * NCH  # (sg, m, ot) awaiting combine+store

            def flush_comb(c):
                """emit combine+store for chunk c (delayed one iteration so
                queue heads never wait on ACT's sigmoid)"""
                rc, route, add_eng, m_eng, comb_eng = CHUNKS[c]
                sg, m, ot = pend[c]
                ENG[comb_eng].tensor_tensor(
                    out=ot, in0=sg, in1=m, op=TT.mult
                )
                sl = slice(offs[c], offs[c] + rc)
                nc.scalar.dma_start(out=outv[:, sl], in_=ot)

            xs[0] = issue_loads(0)
            for c in range(NCH):
                rc, route, add_eng, m_eng, comb_eng = CHUNKS[c]
                if c + 1 < NCH:
                    xs[c + 1] = issue_loads(c + 1)

                x = xs[c]
                v8 = sm_pool.tile([P, rc, 8], f32, tag=f"v8{rc}")
                for r in range(rc):
                    nc.vector.max(out=v8[:, r, :], in_=x[:, r, :])

                # s = v1 + v2  [DVE small]
                s = sm_pool.tile([P, rc], f32, tag=f"s{rc}")
                nc.vector.tensor_tensor(
                    out=s, in0=v8[:, :, 0], in1=v8[:, :, 1], op=TT.add
                )

                # m = (x >= v2b), exact fp32 compare
                m = big_pool.tile([P, rc, E], f32, tag=f"m{rc}", bufs=2)
                ENG[m_eng].tensor_tensor(
                    out=m,
                    in0=x,
                    in1=v8[:, :, 1].to_broadcast([P, rc, E]),
                    op=TT.is_ge,
                )

                sg = big_pool.tile([P, rc, E], f32, tag=f"sg{rc}", bufs=2)
                if route == "A":
                    # sh = s*0.5 on ACT; u = x - shb on GpSimd; one big
                    # sigmoid(2u) on ACT
                    sh = sm_pool.tile([P, rc], f32, tag=f"sh{rc}")
                    nc.scalar.activation(
                        out=sh,
                        in_=s,
                        func=mybir.ActivationFunctionType.Copy,
                        scale=0.5,
                    )
                    u = big_pool.tile([P, rc, E], f32, tag=f"u{rc}", bufs=2)
                    nc.gpsimd.tensor_tensor(
                        out=u,
                        in0=x,
                        in1=sh.to_broadcast([P, rc, E]),
                        op=TT.subtract,
                    )
                    nc.scalar.activation(
                        out=sg,
                        in_=u,
                        func=mybir.ActivationFunctionType.Sigmoid,
                        scale=2.0,
                    )
                else:
                    # negs = -s on ACT; per-row sigmoid(2x - s) with
                    # per-partition bias, all on ACT
                    negs = sm_pool.tile([P, rc], f32, tag=f"ng{rc}")
                    nc.scalar.activation(
                        out=negs,
                        in_=s,
                        func=mybir.ActivationFunctionType.Copy,
                        scale=-1.0,
                    )
                    for r in range(rc):
                        nc.scalar.activation(
                            out=sg[:, r, :],
                            in_=x[:, r, :],
                            func=mybir.ActivationFunctionType.Sigmoid,
                            bias=negs[:, r : r + 1],
                            scale=2.0,
                        )

                ot = big_pool.tile([P, rc, E], f32, tag=f"ot{rc}", bufs=2)
                pend[c] = (sg, m, ot)

                if c >= 1:
                    flush_comb(c - 1)
            flush_comb(NCH - 1)

    _legalize_waits(nc)
    return nc


def _get_nc() -> bass.Bass:
    if "nc" not in _CACHE:
        _CACHE["nc"] = _build_nc()
    return _CACHE["nc"]


def kernel(logits: np.ndarray, noise: np.ndarray) -> np.ndarray:
    global LAST_RESULT
    logits = np.ascontiguousarray(np.asarray(logits), dtype=np.float32)
    noise = np.ascontiguousarray(np.asarray(noise), dtype=np.float32)
    assert logits.shape == (B, E) and noise.shape == (B, E)

    lg_shards = np.split(logits, N_CORES, axis=0)
    nz_shards = np.split(noise, N_CORES, axis=0)
    in_maps = [
        {"logits": lg_shards[i], "noise": nz_shards[i]} for i in range(N_CORES)
    ]

    try:
        res = run_bass_kernel_spmd(
            _get_nc(), in_maps, core_ids=list(range(N_CORES))
        )
    except Exception:
        # transient NRT device errors have been observed right after a
        # crashed run; one retry clears them
        time.sleep(5)
        res = run_bass_kernel_spmd(
            _get_nc(), in_maps, core_ids=list(range(N_CORES))
        )
    LAST_RESULT = res
    return np.concatenate([r["out"] for r in res.results], axis=0)


# revision 11
# speedup vs baseline: 1.0932x; 1.0275x over previous
"""Noisy top-k (k=2) router for Trainium2, data-parallel over 8 NeuronCores.

Math: for each row of x = logits + noise with top-2 values v1 >= v2, the
top-2 softmax weights are sigmoid(+-(v1-v2)), and for any element
    out = (x >= v2) * sigmoid(2x - (v1+v2))
equals the scattered result exactly (selection compares run on exact fp32
values, so this matches the reference whenever no row has a v2 == v3 tie).

Engine split (v4, tuned against measured cadences: MAX8 ~128ns/row c2c,
DVE full pass ~1.1ns/elem, GpSimd TT ~2.2ns/elem, ACT ~0.85ns/elem big /
~333ns per per-row ACTIVATE):
  - DVE runs ONLY the per-row MAX8 stream + s=v1+v2 + the exact is_ge
    mask (plus a few ramp/tail combines): ~60us/core, the critical path.
    It must never wait: loads+adds run 2 chunks ahead; nothing DVE emits
    depends on ACT/GpSimd results except the delayed combines.
  - sigmoid hybrid per chunk: route A: u = x - (v1+v2)/2 on GpSimd then
    one big ACT sigmoid(scale=2); route B: per-row ACT sigmoid with
    bias=-(v1+v2), no u pass. Mix balances ACT vs GpSimd.
  - combine out = sig * mask on GpSimd (DVE on ramp/tail chunks),
    delayed one chunk so queue heads never wait on the sigmoid.
  - adds via SWDGE accum-DMA (CCE) issued 2 chunks ahead; chunk 0 adds
    on DVE. Loads on the SP HWDGE ring, stores on the ACT ring, delayed
    two chunks.
This walrus codegen allows only ONE sync-wait per instruction; the
_legalize_waits post-pass splits any excess into standalone EventSemaphore
instructions (which hold two).
"""

import time

import numpy as np

import concourse.bass as bass
import concourse.mybir as mybir
from concourse.tile import TileContext
from concourse.bass_utils import run_bass_kernel_spmd

B = 262144
E = 64
N_CORES = 8
B_CORE = B // N_CORES  # 32768 rows per core

P = 128  # SBUF partitions
ROWS_PP = B_CORE // P  # 256 rows per partition

RC = 16  # rows per partition per chunk
NCH = ROWS_PP // RC  # 16 chunks

# per-chunk sigmoid route: 'A' = u-pass + one big ACT sigmoid,
# 'B' = per-row biased ACT sigmoid (heavier on ACT, nothing on GpSimd)
ROUTES = "BBABAAABAAABAAAB"
assert len(ROUTES) == NCH and ROUTES.count("B") == 6

# chunks whose combine runs on DVE (ramp/tail, where DVE has slack)
COMB_DVE = (0, 1, 14, 15)
# chunks whose adds run as a DVE TT (ramp; SWDGE has ~1us extra latency)
ADD_DVE = (0,)

_CACHE = {}

# test.py introspection: BassKernelResults of the most recent run
LAST_RESULT = None


def _legalize_waits(nc: "bass.Bass") -> None:
    """This walrus codegen accepts at most ONE sync-wait per instruction
    (two on EventSemaphore). Tile's wait assigner can emit more; split the
    excess into standalone EventSemaphore waits placed immediately before
    the instruction on the same engine (identical semantics: the engine
    blocks there instead)."""
    n = 0
    for fnb in nc.m.functions[0].blocks:
        out = []
        for inst in fnb.instructions:
            si = inst.sync_info
            cap = 2 if isinstance(inst, mybir.InstEventSemaphore) else 1
            if si is not None and len(si.on_wait) > cap:
                waits = list(si.on_wait)
                extra, keep = waits[:-cap], waits[-cap:]
                for c in range(0, len(extra), 2):
                    n += 1
                    out.append(
                        mybir.InstEventSemaphore(
                            name=f"EVW-{n}",
                            engine=inst.engine,
                            sync_info=mybir.SyncInfo(
                                on_wait=extra[c : c + 2], on_update=[]
                            ),
                        )
                    )
                inst.sync_info = mybir.SyncInfo(
                    on_wait=keep, on_update=list(si.on_update)
                )
            out.append(inst)
        fnb.instructions = out


def _build_nc() -> bass.Bass:
    nc = bass.Bass()
    f32 = mybir.dt.float32
    TT = mybir.AluOpType
    AF = mybir.ActivationFunctionType

    lg = nc.dram_tensor("logits", [B_CORE, E], f32, kind="ExternalInput")
    nz = nc.dram_tensor("noise", [B_CORE, E], f32, kind="ExternalInput")
    out = nc.dram_tensor("out", [B_CORE, E], f32, kind="ExternalOutput")

    # partition-major: partition p owns ROWS_PP contiguous DRAM rows; a
    # chunk is one 4KB-contiguous descriptor per partition
    lgv = lg[:].rearrange("(p c r) e -> c p r e", p=P, c=NCH)
    nzv = nz[:].rearrange("(p c r) e -> c p r e", p=P, c=NCH)
    outv = out[:].rearrange("(p c r) e -> c p r e", p=P, c=NCH)

    with TileContext(nc) as tc:
        with (
            tc.tile_pool(name="big", bufs=4) as big_pool,
            tc.tile_pool(name="sm", bufs=4) as sm_pool,
        ):

            def issue_loads(c):
                x = big_pool.tile([P, RC, E], f32, tag="x", bufs=6)
                if c in ADD_DVE:
                    lgt = big_pool.tile([P, RC, E], f32, tag="lgt", bufs=1)
                    nzt = big_pool.tile([P, RC, E], f32, tag="nzt", bufs=1)
                    nc.sync.dma_start(out=lgt, in_=lgv[c])
                    nc.sync.dma_start(out=nzt, in_=nzv[c])
                    nc.vector.tensor_tensor(
                        out=x, in0=lgt, in1=nzt, op=TT.add
                    )
                else:
                    nc.sync.dma_start(out=x, in_=lgv[c])
                    nc.gpsimd.dma_start(
                        out=x, in_=nzv[c], accum_op=TT.add
                    )
                return x

            xs = [None] * NCH
            pend = [None] * NCH  # (sg, m, ot) awaiting combine
            done = [None] * NCH  # ot awaiting store

            def emit_comb(c):
                sg, m, ot = pend[c]
                eng = nc.vector if c in COMB_DVE else nc.gpsimd
                eng.tensor_tensor(out=ot, in0=sg, in1=m, op=TT.mult)
                done[c] = ot

            def emit_store(c):
                nc.scalar.dma_start(out=outv[c], in_=done[c])

            xs[0] = issue_loads(0)
            xs[1] = issue_loads(1)
            for c in range(NCH):
                if c + 2 < NCH:
                    xs[c + 2] = issue_loads(c + 2)

                x = xs[c]
                v8 = sm_pool.tile([P, RC, 8], f32, tag="v8")
                for r in range(RC):
                    nc.vector.max(out=v8[:, r, :], in_=x[:, r, :])

                # s = v1 + v2  [DVE small]
                s = sm_pool.tile([P, RC], f32, tag="s")
                nc.vector.tensor_tensor(
                    out=s, in0=v8[:, :, 0], in1=v8[:, :, 1], op=TT.add
                )

                sg = big_pool.tile([P, RC, E], f32, tag="sg")
                if ROUTES[c] == "A":
                    # sh = s*0.5 (ACT); u = x - shb (GpSimd); big sigmoid
                    sh = sm_pool.tile([P, RC], f32, tag="sh")
                    nc.scalar.activation(
                        out=sh, in_=s, func=AF.Copy, scale=0.5
                    )
                    u = big_pool.tile([P, RC, E], f32, tag="u")
                    nc.gpsimd.tensor_tensor(
                        out=u,
                        in0=x,
                        in1=sh.to_broadcast([P, RC, E]),
                        op=TT.subtract,
                    )
                    nc.scalar.activation(
                        out=sg, in_=u, func=AF.Sigmoid, scale=2.0
                    )
                else:
                    # negs = -s (ACT); per-row sigmoid(2x - s), all on ACT
                    negs = sm_pool.tile([P, RC], f32, tag="ng")
                    nc.scalar.activation(
                        out=negs, in_=s, func=AF.Copy, scale=-1.0
                    )
                    for r in range(RC):
                        nc.scalar.activation(
                            out=sg[:, r, :],
                            in_=x[:, r, :],
                            func=AF.Sigmoid,
                            bias=negs[:, r : r + 1],
                            scale=2.0,
                        )

                # m = (x >= v2b), exact fp32 compare  [DVE]
                m = big_pool.tile([P, RC, E], f32, tag="m")
                nc.vector.tensor_tensor(
                    out=m,
                    in0=x,
                    in1=v8[:, :, 1].to_broadcast([P, RC, E]),
                    op=TT.is_ge,
                )

                ot = big_pool.tile([P, RC, E], f32, tag="ot")
                pend[c] = (sg, m, ot)

                if c >= 1:
                    emit_comb(c - 1)
                if c >= 2:
                    emit_store(c - 2)
            emit_comb(NCH - 1)
            emit_store(NCH - 2)
            emit_store(NCH - 1)

    _legalize_waits(nc)
    return nc


def _get_nc() -> bass.Bass:
    if "nc" not in _CACHE:
        _CACHE["nc"] = _build_nc()
    return _CACHE["nc"]


def kernel(logits: np.ndarray, noise: np.ndarray) -> np.ndarray:
    global LAST_RESULT
    logits = np.ascontiguousarray(np.asarray(logits), dtype=np.float32)
    noise = np.ascontiguousarray(np.asarray(noise), dtype=np.float32)
    assert logits.shape == (B, E) and noise.shape == (B, E)

    lg_shards = np.split(logits, N_CORES, axis=0)
    nz_shards = np.split(noise, N_CORES, axis=0)
    in_maps = [
        {"logits": lg_shards[i], "noise": nz_shards[i]} for i in range(N_CORES)
    ]

    try:
        res = run_bass_kernel_spmd(
            _get_nc(), in_maps, core_ids=list(range(N_CORES))
        )
    except Exception:
        # transient NRT device errors have been observed right after a
        # crashed run; one retry clears them
        time.sleep(5)
        res = run_bass_kernel_spmd(
            _get_nc(), in_maps, core_ids=list(range(N_CORES))
        )
    LAST_RESULT = res
    return np.concatenate([r["out"] for r in res.results], axis=0)
